# revision 10
# baseline (speedup 1.0000x reference)
"""Trainium2 Bass kernel for BackgroundSubtractorModule.

Reference computation (per 15-frame window, gray video):
  y      = 0.299 R + 0.587 G + 0.114 B            (per pixel, x scale)
  m      = mean_f y ; var = sum_f (y-m)^2 / 14
  sigma  = sqrt(var) + 1e-5
  bg     = |y - m| / sigma
  minv/maxv = min/max over pixels of bg (per frame)
  out    = (bg - minv) / (maxv - minv)  if rng > 1e-6 else bg

Sharding: 30 independent windows across 8 cores; every core runs an
identical 4-window program (cores 6,7 process one duplicated pad window
whose output is dropped).

Design (v3 — fp16 + fused ops, trace-driven):
  * minv of bg over 147456 pixels is ~1e-5 while rng ~4; dropping it
    contributes ~2e-6 rel error: out = |bg| / maxv.
  * Everything after the rgb load runs in fp16 (measured end-to-end
    4.1e-3 vs the 2e-2 tolerance); fp16 stores halve output traffic.
  * Luma in two fused scalar_tensor_tensor ops: t2 = (B*a_b)+G on
    GPSIMD, y = (R*a_r)+t2 on DVE — no separate copies or adds.
  * Squared-domain tail keeps everything sign-free so plain max works:
    d = y-m; d2 = d*d; u = d2*inv_sigma2 = bg^2 via tensor_tensor_reduce
    whose accum_out yields per-frame max(u) = maxv^2 in the same pass.
    Final normalize is ACT Sqrt(u * inv_maxv2) = |bg|/maxv.
  * PE accumulates sum(y) and sum(y^2) as fp16 identity-matmul PSUM
    accumulation; ACT squares y (3-frame batched).
  * 3-frame batched loads (5.3 MB/DMA) on sync HWDGE; 3-frame fp16
    stores on scalar HWDGE.
"""

import numpy as np
from contextlib import ExitStack

import concourse.bass as bass
import concourse.bacc as bacc
import concourse.tile as tile
from concourse import mybir, bass_isa
from concourse.bass_utils import run_bass_kernel_spmd

F32 = mybir.dt.float32
F16 = mybir.dt.float16
OP = mybir.AluOpType
AF = mybir.ActivationFunctionType

T, H, W = 450, 384, 384
PIX = H * W                    # 147456
WIN = 15
NCORES = 8
NWIN_CORE = 4                  # ceil(30/8) -> uniform SPMD program
FPC = NWIN_CORE * WIN          # 60 frames per core
P = 128
COLS = PIX // P                # 1152
EPS = 1e-5
BANKS = ((0, 512), (512, 1024), (1024, 1152))   # PSUM bank-aligned slices

_BUILD_CACHE = {}


def _build(scale: float):
    w0, w1, w2 = 0.299 * scale, 0.587 * scale, 0.114 * scale
    a_r, a_b = w0 / w1, w2 / w1
    nc = bacc.Bacc("TRN2", target_bir_lowering=False, debug=False)
    vin = nc.dram_tensor("video", [FPC, PIX * 3], F32, kind="ExternalInput").ap()
    idd = nc.dram_tensor("ident", [P, P], F16, kind="ExternalInput").ap()
    vout = nc.dram_tensor("out", [FPC, PIX], F16, kind="ExternalOutput").ap()

    with tile.TileContext(nc) as tc, ExitStack() as ctx:
        p_const = ctx.enter_context(tc.tile_pool(name="const", bufs=1))
        p_y = ctx.enter_context(tc.tile_pool(name="y", bufs=2))
        p_rgb = ctx.enter_context(tc.tile_pool(name="rgb", bufs=2))
        p_stat = ctx.enter_context(tc.tile_pool(name="stat", bufs=2))
        p_tmp = ctx.enter_context(tc.tile_pool(name="tmp", bufs=1))
        p_ftmp = ctx.enter_context(tc.tile_pool(name="ftmp", bufs=2))
        p_mm = ctx.enter_context(tc.tile_pool(name="mm", bufs=2))
        p_ps = ctx.enter_context(tc.tile_pool(name="psum", bufs=1, space="PSUM"))

        ident = p_const.tile([P, P], F16)
        nc.sync.dma_start(ident[:], idd[:])

        st8 = {}

        def mk_state(w):
            st8[w] = dict(
                yt=p_y.tile([P, WIN * COLS], F16, tag="y", name=f"yt{w}"),
                acc_s=p_ps.tile([P, COLS], F32, tag="acc_s", name=f"accs{w}"),
                acc_q=p_ps.tile([P, COLS], F32, tag="acc_q", name=f"accq{w}"),
                mt=p_stat.tile([P, COLS], F16, tag="m", name=f"mt{w}"),
                ish=p_stat.tile([P, COLS], F16, tag="ish", name=f"ish{w}"),
                mmt=p_mm.tile([P, 48], F32, tag="mm", name=f"mmt{w}"),
            )
            nc.gpsimd.memset(st8[w]["mmt"][:, 0:16], 0.0)

        def yslice(w, f):
            yt = st8[w]["yt"]
            return yt[:, f * COLS:(f + 1) * COLS]

        def p1_group(w, grp):
            """Load 3 frames, fused luma, square, PE accumulation."""
            S = st8[w]
            g = w * WIN + grp * 3
            f0 = grp * 3
            rgbt = p_rgb.tile([P, 3 * COLS * 3], F32, tag="rgb")
            nc.sync.dma_start(
                rgbt[:].rearrange("p (f x) -> p f x", f=3),
                vin[g:g + 3].rearrange("f (r x) -> r f x", r=P))
            rgb4 = rgbt[:].rearrange("p (f j c) -> p f j c", f=3, c=3)
            yg = S["yt"][:, f0 * COLS:(f0 + 3) * COLS]
            yg3 = yg.rearrange("p (f j) -> p f j", f=3)
            t2 = p_ftmp.tile([P, 3 * COLS], F16, tag="t2")
            t23 = t2[:].rearrange("p (f j) -> p f j", f=3)
            # t2 = B * a_b (ACT), then t2 += G (GPSIMD)
            nc.scalar.activation(t23, rgb4[:, :, :, 2], AF.Copy,
                                 bias=0.0, scale=a_b)
            nc.gpsimd.tensor_tensor(t23, t23, rgb4[:, :, :, 1], OP.add)
            # y = (R * a_r) + t2     (DVE fused, fp16 out)
            nc.vector.scalar_tensor_tensor(
                yg3, rgb4[:, :, :, 0], a_r, t23, OP.mult, OP.add)
            sq = p_ftmp.tile([P, 3 * COLS], F16, tag="sq")
            nc.scalar.activation(sq[:], yg, AF.Square)
            for k in range(3):
                f = f0 + k
                yf = yslice(w, f)
                sqf = sq[:, k * COLS:(k + 1) * COLS]
                for lo, hi in BANKS:
                    nc.tensor.matmul(S["acc_s"][:, lo:hi], ident[:], yf[:, lo:hi],
                                     start=(f == 0), stop=(f == WIN - 1))
                for lo, hi in BANKS:
                    nc.tensor.matmul(S["acc_q"][:, lo:hi], ident[:], sqf[:, lo:hi],
                                     start=(f == 0), stop=(f == WIN - 1))

        def p2(w):
            S = st8[w]
            # mean (fp16, for the subtract)
            nc.vector.tensor_scalar(S["mt"][:], S["acc_s"][:], 1.0 / WIN, None,
                                    OP.mult)
            # 15*m^2 = (sum/sqrt(15))^2, exact from psum
            msq = p_tmp.tile([P, COLS], F32, tag="msq")
            nc.scalar.activation(msq[:], S["acc_s"][:], AF.Square,
                                 scale=float(1.0 / np.sqrt(15.0)))
            vs = p_tmp.tile([P, COLS], F32, tag="vs")
            nc.vector.tensor_tensor(vs[:], S["acc_q"][:], msq[:], OP.subtract)
            # sigma' = sqrt(varsum/14); inv_s2 = (1/(sigma' + EPS/w1))^2
            nc.scalar.activation(vs[:], vs[:], AF.Sqrt,
                                 scale=float(1.0 / (WIN - 1)))
            nc.vector.tensor_scalar(vs[:], vs[:], float(EPS / w1), None, OP.add)
            scr = p_tmp.tile([P, COLS], F32, tag="msq")
            nc.vector.reciprocal_approx_accurate(vs[:], vs[:], scr[:])
            nc.scalar.activation(S["ish"][:], vs[:], AF.Square)

        def p3_group(w, grp):
            S = st8[w]
            f0 = grp * 3
            for f in range(f0, f0 + 3):
                nc.vector.tensor_tensor(yslice(w, f), yslice(w, f), S["mt"][:],
                                        OP.subtract)             # d, fp16 2x
            for f in range(f0, f0 + 3):
                nc.vector.tensor_tensor(yslice(w, f), yslice(w, f), yslice(w, f),
                                        OP.mult)                 # d^2
            for f in range(f0, f0 + 3):
                # u = d^2 * inv_s2 = bg^2
                nc.vector.tensor_tensor(yslice(w, f), yslice(w, f), S["ish"][:],
                                        OP.mult)
            ych3 = S["yt"][:, f0 * COLS:(f0 + 3) * COLS].rearrange(
                "p (f j) -> p f j", f=3)
            nc.vector.tensor_reduce(
                S["mmt"][:, f0:f0 + 3], ych3, axis=mybir.AxisListType.X,
                op=OP.max)                                       # maxv^2

        def p4(w):
            mmt = st8[w]["mmt"]
            nc.gpsimd.partition_all_reduce(
                mmt[:, 16:32], mmt[:, 0:16], 128, bass_isa.ReduceOp.max)
            nc.vector.reciprocal(mmt[:, 32:48], mmt[:, 16:32])

        def p5_group(w, grp):
            S = st8[w]
            mmt = S["mmt"]
            f0 = grp * 3
            for f in range(f0, f0 + 3):
                # out = sqrt(u * inv_maxv2) = |bg| / maxv
                nc.scalar.activation(
                    yslice(w, f), yslice(w, f), AF.Sqrt,
                    bias=0.0, scale=mmt[:, 32 + f:33 + f])
            g0 = w * WIN + f0
            nc.scalar.dma_start(
                vout[g0:g0 + 3].rearrange("f (r j) -> r f j", r=P),
                S["yt"][:, f0 * COLS:(f0 + 3) * COLS].rearrange(
                    "p (f j) -> p f j", f=3),
            )

        # ---- software-pipelined emission ----
        mk_state(0)
        for grp in range(5):
            p1_group(0, grp)
        for w in range(NWIN_CORE):
            nxt = w + 1 if w + 1 < NWIN_CORE else None
            if nxt is not None:
                mk_state(nxt)
            p2(w)
            for grp in range(5):
                p3_group(w, grp)
                if nxt is not None and grp < 3:
                    p1_group(nxt, grp)
            p4(w)
            for grp in range(5):
                p5_group(w, grp)
                if nxt is not None and grp < 2:
                    p1_group(nxt, 3 + grp)
            del st8[w]

    nc.compile()
    return nc


def _get_nc(scale: float):
    key = round(float(scale), 9)
    if key not in _BUILD_CACHE:
        _BUILD_CACHE[key] = _build(key)
    return _BUILD_CACHE[key]


def kernel(video: np.ndarray) -> np.ndarray:
    video = np.ascontiguousarray(np.asarray(video, dtype=np.float32))
    assert video.shape == (T, H, W, 3), video.shape
    scale = 1.0 / 255.0 if float(video.max()) > 1.0 else 1.0
    nc = _get_nc(scale)

    v = video.reshape(T, PIX * 3)
    shards = []
    for c in range(6):
        shards.append(v[c * FPC:(c + 1) * FPC])
    # cores 6,7: 3 real windows + last window repeated as pad
    shards.append(np.concatenate([v[360:405], v[390:405]], axis=0))
    shards.append(np.concatenate([v[405:450], v[435:450]], axis=0))

    ident = np.eye(P, dtype=np.float16)
    res = run_bass_kernel_spmd(
        nc, [{"video": s, "ident": ident} for s in shards], list(range(NCORES))
    )
    outs = [res.results[c]["out"].astype(np.float32) for c in range(NCORES)]
    full = np.concatenate(
        [o[:FPC] for o in outs[:6]] + [outs[6][:45], outs[7][:45]], axis=0
    )
    return full.reshape(T, 1, H, W)


# revision 11
# speedup vs baseline: 1.1074x; 1.1074x over previous
"""Trainium2 Bass kernel for BackgroundSubtractorModule.

Reference computation (per 15-frame window, gray video):
  y      = 0.299 R + 0.587 G + 0.114 B            (per pixel, x scale)
  m      = mean_f y ; var = sum_f (y-m)^2 / 14
  sigma  = sqrt(var) + 1e-5
  bg     = |y - m| / sigma
  minv/maxv = min/max over pixels of bg (per frame)
  out    = (bg - minv) / (maxv - minv)  if rng > 1e-6 else bg

Sharding: 30 independent windows across 8 cores; every core runs an
identical 4-window program (cores 6,7 process one duplicated pad window
whose output is dropped).

Design (v4 — fp16 pipeline, per-frame ops, deep prefetch):
  * minv of bg over 147456 pixels is ~1e-5 while rng ~4; dropping it
    contributes ~2e-6 rel error: out = |bg| / maxv.
  * Everything after the rgb load runs in fp16 (measured end-to-end
    ~4e-3 vs the 2e-2 tolerance); fp16 stores halve output traffic.
  * Luma: ACT makes the two scaled strided copies (fp16 out), GPSIMD
    adds G, DVE adds the halves (fp16 2x). Square split ACT/DVE by knob.
  * PE accumulates sum(y) and sum(y^2) as fp16 identity-matmul PSUM
    accumulation (6 banks).
  * abs fused into the per-frame max reduce (apply_absolute_value);
    normalize is ACT Abs(bg * inv_maxv) with a per-partition scale col.
  * Per-frame 1.77 MB loads on sync HWDGE with a 5-buffer ring for deep
    prefetch; 3-frame fp16 stores on scalar HWDGE.
"""

import numpy as np
from contextlib import ExitStack

import concourse.bass as bass
import concourse.bacc as bacc
import concourse.tile as tile
from concourse import mybir, bass_isa
from concourse.bass_utils import run_bass_kernel_spmd

F32 = mybir.dt.float32
F16 = mybir.dt.float16
OP = mybir.AluOpType
AF = mybir.ActivationFunctionType

T, H, W = 450, 384, 384
PIX = H * W                    # 147456
WIN = 15
NCORES = 8
NWIN_CORE = 4                  # ceil(30/8) -> uniform SPMD program
FPC = NWIN_CORE * WIN          # 60 frames per core
P = 128
COLS = PIX // P                # 1152
EPS = 1e-5
BANKS = ((0, 512), (512, 1024), (1024, 1152))   # PSUM bank-aligned slices

# engine-balance knobs (frame index within window)
SQ_ON_DVE = lambda f: f % 3 != 0   # square: 10 frames DVE, 5 frames ACT

_BUILD_CACHE = {}


def _build(scale: float):
    w0, w1, w2 = 0.299 * scale, 0.587 * scale, 0.114 * scale
    a_r, a_b = w0 / w1, w2 / w1
    nc = bacc.Bacc("TRN2", target_bir_lowering=False, debug=False)
    vin = nc.dram_tensor("video", [FPC, PIX * 3], F32, kind="ExternalInput").ap()
    idd = nc.dram_tensor("ident", [P, P], F16, kind="ExternalInput").ap()
    vout = nc.dram_tensor("out", [FPC, PIX], F16, kind="ExternalOutput").ap()

    with tile.TileContext(nc) as tc, ExitStack() as ctx:
        p_const = ctx.enter_context(tc.tile_pool(name="const", bufs=1))
        p_y = ctx.enter_context(tc.tile_pool(name="y", bufs=2))
        p_rgb = ctx.enter_context(tc.tile_pool(name="rgb", bufs=5))
        p_stat = ctx.enter_context(tc.tile_pool(name="stat", bufs=2))
        p_tmp = ctx.enter_context(tc.tile_pool(name="tmp", bufs=1))
        p_ftmp = ctx.enter_context(tc.tile_pool(name="ftmp", bufs=3))
        p_mm = ctx.enter_context(tc.tile_pool(name="mm", bufs=2))
        p_ps = ctx.enter_context(tc.tile_pool(name="psum", bufs=1, space="PSUM"))

        ident = p_const.tile([P, P], F16)
        nc.sync.dma_start(ident[:], idd[:])

        st8 = {}

        def mk_state(w):
            st8[w] = dict(
                yt=p_y.tile([P, WIN * COLS], F16, tag="y", name=f"yt{w}"),
                acc_s=p_ps.tile([P, COLS], F32, tag="acc_s", name=f"accs{w}"),
                acc_q=p_ps.tile([P, COLS], F32, tag="acc_q", name=f"accq{w}"),
                mt=p_stat.tile([P, COLS], F16, tag="m", name=f"mt{w}"),
                ish=p_stat.tile([P, COLS], F16, tag="ish", name=f"ish{w}"),
                mmt=p_mm.tile([P, 48], F32, tag="mm", name=f"mmt{w}"),
            )
            nc.gpsimd.memset(st8[w]["mmt"][:, 0:16], 0.0)

        def yslice(w, f):
            yt = st8[w]["yt"]
            return yt[:, f * COLS:(f + 1) * COLS]

        def p1_frame(w, f):
            S = st8[w]
            g = w * WIN + f
            rgbt = p_rgb.tile([P, COLS * 3], F32, tag="rgb")
            nc.sync.dma_start(rgbt[:], vin[g].rearrange("(r j) -> r j", r=P))
            rgb3 = rgbt[:].rearrange("p (j c) -> p j c", c=3)
            yf = yslice(w, f)
            t2 = p_ftmp.tile([P, COLS], F16, tag="t2")
            nc.scalar.activation(yf, rgb3[:, :, 0], AF.Copy, bias=0.0, scale=a_r)
            nc.scalar.activation(t2[:], rgb3[:, :, 2], AF.Copy, bias=0.0, scale=a_b)
            nc.gpsimd.tensor_tensor(t2[:], t2[:], rgb3[:, :, 1], OP.add)
            nc.vector.tensor_tensor(yf, yf, t2[:], OP.add)       # fp16 2x
            sq = p_ftmp.tile([P, COLS], F16, tag="sq")
            if SQ_ON_DVE(f):
                nc.vector.tensor_tensor(sq[:], yf, yf, OP.mult)  # fp16 2x
            else:
                nc.scalar.activation(sq[:], yf, AF.Square)
            for lo, hi in BANKS:
                nc.tensor.matmul(S["acc_s"][:, lo:hi], ident[:], yf[:, lo:hi],
                                 start=(f == 0), stop=(f == WIN - 1))
            for lo, hi in BANKS:
                nc.tensor.matmul(S["acc_q"][:, lo:hi], ident[:], sq[:, lo:hi],
                                 start=(f == 0), stop=(f == WIN - 1))

        def p2(w):
            S = st8[w]
            # mean (fp16, for the subtract)
            nc.vector.tensor_scalar(S["mt"][:], S["acc_s"][:], 1.0 / WIN, None,
                                    OP.mult)
            # 15*m^2 = (sum/sqrt(15))^2, exact from psum
            msq = p_tmp.tile([P, COLS], F32, tag="msq")
            nc.scalar.activation(msq[:], S["acc_s"][:], AF.Square,
                                 scale=float(1.0 / np.sqrt(15.0)))
            vs = p_tmp.tile([P, COLS], F32, tag="vs")
            nc.vector.tensor_tensor(vs[:], S["acc_q"][:], msq[:], OP.subtract)
            # sigma' = sqrt(varsum/14); inv_s = 1/(sigma' + EPS/w1)
            nc.scalar.activation(vs[:], vs[:], AF.Sqrt,
                                 scale=float(1.0 / (WIN - 1)))
            nc.vector.tensor_scalar(vs[:], vs[:], float(EPS / w1), None, OP.add)
            scr = p_tmp.tile([P, COLS], F32, tag="msq")
            nc.vector.reciprocal_approx_accurate(vs[:], vs[:], scr[:])
            nc.vector.tensor_copy(S["ish"][:], vs[:])            # cast -> fp16

        def p3_group(w, grp):
            S = st8[w]
            f0 = grp * 3
            for f in range(f0, f0 + 3):
                nc.vector.tensor_tensor(yslice(w, f), yslice(w, f), S["mt"][:],
                                        OP.subtract)             # d, fp16 2x
            for f in range(f0, f0 + 3):
                nc.vector.tensor_tensor(yslice(w, f), yslice(w, f), S["ish"][:],
                                        OP.mult)                 # bg signed
            ych3 = S["yt"][:, f0 * COLS:(f0 + 3) * COLS].rearrange(
                "p (f j) -> p f j", f=3)
            nc.vector.tensor_reduce(
                S["mmt"][:, f0:f0 + 3], ych3, axis=mybir.AxisListType.X,
                op=OP.max, apply_absolute_value=True)            # max |bg|

        def p4(w):
            mmt = st8[w]["mmt"]
            nc.gpsimd.partition_all_reduce(
                mmt[:, 16:32], mmt[:, 0:16], 128, bass_isa.ReduceOp.max)
            nc.vector.reciprocal(mmt[:, 32:48], mmt[:, 16:32])

        def p5_group(w, grp):
            S = st8[w]
            mmt = S["mmt"]
            f0 = grp * 3
            for f in range(f0, f0 + 3):
                # |bg * inv_maxv| : ACT Abs with per-partition scale column
                nc.scalar.activation(
                    yslice(w, f), yslice(w, f), AF.Abs,
                    bias=0.0, scale=mmt[:, 32 + f:33 + f])
            g0 = w * WIN + f0
            nc.scalar.dma_start(
                vout[g0:g0 + 3].rearrange("f (r j) -> r f j", r=P),
                S["yt"][:, f0 * COLS:(f0 + 3) * COLS].rearrange(
                    "p (f j) -> p f j", f=3),
            )

        # ---- software-pipelined emission ----
        mk_state(0)
        for f in range(WIN):
            p1_frame(0, f)
        for w in range(NWIN_CORE):
            nxt = w + 1 if w + 1 < NWIN_CORE else None
            if nxt is not None:
                mk_state(nxt)
            p2(w)
            for grp in range(5):
                p3_group(w, grp)
                if nxt is not None:
                    p1_frame(nxt, grp * 2)
                    p1_frame(nxt, grp * 2 + 1)
            p4(w)
            for grp in range(5):
                p5_group(w, grp)
                if nxt is not None and 10 + grp < WIN:
                    p1_frame(nxt, 10 + grp)
            del st8[w]

    nc.compile()
    return nc


def _get_nc(scale: float):
    key = round(float(scale), 9)
    if key not in _BUILD_CACHE:
        _BUILD_CACHE[key] = _build(key)
    return _BUILD_CACHE[key]


def kernel(video: np.ndarray) -> np.ndarray:
    video = np.ascontiguousarray(np.asarray(video, dtype=np.float32))
    assert video.shape == (T, H, W, 3), video.shape
    scale = 1.0 / 255.0 if float(video.max()) > 1.0 else 1.0
    nc = _get_nc(scale)

    v = video.reshape(T, PIX * 3)
    shards = []
    for c in range(6):
        shards.append(v[c * FPC:(c + 1) * FPC])
    # cores 6,7: 3 real windows + last window repeated as pad
    shards.append(np.concatenate([v[360:405], v[390:405]], axis=0))
    shards.append(np.concatenate([v[405:450], v[435:450]], axis=0))

    ident = np.eye(P, dtype=np.float16)
    res = run_bass_kernel_spmd(
        nc, [{"video": s, "ident": ident} for s in shards], list(range(NCORES))
    )
    outs = [res.results[c]["out"].astype(np.float32) for c in range(NCORES)]
    full = np.concatenate(
        [o[:FPC] for o in outs[:6]] + [outs[6][:45], outs[7][:45]], axis=0
    )
    return full.reshape(T, 1, H, W)


# revision 12
# speedup vs baseline: 1.3902x; 1.2554x over previous
"""Trainium2 Bass kernel for BackgroundSubtractorModule.

Reference computation (per 15-frame window, gray video):
  y      = 0.299 R + 0.587 G + 0.114 B            (per pixel, x scale)
  m      = mean_f y ; var = sum_f (y-m)^2 / 14
  sigma  = sqrt(var) + 1e-5
  bg     = |y - m| / sigma
  minv/maxv = min/max over pixels of bg (per frame)
  out    = (bg - minv) / (maxv - minv)  if rng > 1e-6 else bg

Sharding: 30 independent windows across 8 cores; every core runs an
identical 4-window program (cores 6,7 process one duplicated pad window
whose output is dropped).

Design (v4 — fp16 pipeline, per-frame ops, deep prefetch):
  * minv of bg over 147456 pixels is ~1e-5 while rng ~4; dropping it
    contributes ~2e-6 rel error: out = |bg| / maxv.
  * Everything after the rgb load runs in fp16 (measured end-to-end
    ~4e-3 vs the 2e-2 tolerance); fp16 stores halve output traffic.
  * Luma: ACT makes the two scaled strided copies (fp16 out), GPSIMD
    adds G, DVE adds the halves (fp16 2x). Square split ACT/DVE by knob.
  * PE accumulates sum(y) and sum(y^2) as fp16 identity-matmul PSUM
    accumulation (6 banks).
  * abs fused into the per-frame max reduce (apply_absolute_value);
    normalize is ACT Abs(bg * inv_maxv) with a per-partition scale col.
  * Per-frame 1.77 MB loads on sync HWDGE with a 5-buffer ring for deep
    prefetch; 3-frame fp16 stores on scalar HWDGE.
"""

import numpy as np
from contextlib import ExitStack

import concourse.bass as bass
import concourse.bacc as bacc
import concourse.tile as tile
from concourse import mybir, bass_isa
from concourse.bass_utils import run_bass_kernel_spmd

F32 = mybir.dt.float32
F16 = mybir.dt.float16
OP = mybir.AluOpType
AF = mybir.ActivationFunctionType

T, H, W = 450, 384, 384
PIX = H * W                    # 147456
WIN = 15
NCORES = 8
NWIN_CORE = 4                  # ceil(30/8) -> uniform SPMD program
FPC = NWIN_CORE * WIN          # 60 frames per core
P = 128
COLS = PIX // P                # 1152
EPS = 1e-5
BANKS = ((0, 512), (512, 1024), (1024, 1152))   # PSUM bank-aligned slices

# engine-balance knobs (frame index within window)
SQ_ON_DVE = lambda f: f % 3 != 0   # square: 10 frames DVE, 5 frames ACT

_BUILD_CACHE = {}


def _build(scale: float):
    w0, w1, w2 = 0.299 * scale, 0.587 * scale, 0.114 * scale
    a_r, a_b = w0 / w1, w2 / w1
    nc = bacc.Bacc("TRN2", target_bir_lowering=False, debug=False)
    vin = nc.dram_tensor("video", [FPC, PIX * 3], F32, kind="ExternalInput").ap()
    idd = nc.dram_tensor("ident", [P, P], F16, kind="ExternalInput").ap()
    vout = nc.dram_tensor("out", [FPC, PIX], F16, kind="ExternalOutput").ap()

    with tile.TileContext(nc) as tc, ExitStack() as ctx:
        p_const = ctx.enter_context(tc.tile_pool(name="const", bufs=1))
        p_y = ctx.enter_context(tc.tile_pool(name="y", bufs=2))
        p_rgb = ctx.enter_context(tc.tile_pool(name="rgb", bufs=2))
        p_stat = ctx.enter_context(tc.tile_pool(name="stat", bufs=2))
        p_tmp = ctx.enter_context(tc.tile_pool(name="tmp", bufs=1))
        p_ftmp = ctx.enter_context(tc.tile_pool(name="ftmp", bufs=3))
        p_mm = ctx.enter_context(tc.tile_pool(name="mm", bufs=2))
        p_ps = ctx.enter_context(tc.tile_pool(name="psum", bufs=1, space="PSUM"))

        ident = p_const.tile([P, P], F16)
        nc.sync.dma_start(ident[:], idd[:])

        st8 = {}

        def mk_state(w):
            st8[w] = dict(
                yt=p_y.tile([P, WIN * COLS], F16, tag="y", name=f"yt{w}"),
                acc_s=p_ps.tile([P, COLS], F32, tag="acc_s", name=f"accs{w}"),
                acc_q=p_ps.tile([P, COLS], F32, tag="acc_q", name=f"accq{w}"),
                mt=p_stat.tile([P, COLS], F16, tag="m", name=f"mt{w}"),
                ish=p_stat.tile([P, COLS], F16, tag="ish", name=f"ish{w}"),
                mmt=p_mm.tile([P, 48], F32, tag="mm", name=f"mmt{w}"),
            )
            nc.gpsimd.memset(st8[w]["mmt"][:, 0:16], 0.0)

        def yslice(w, f):
            yt = st8[w]["yt"]
            return yt[:, f * COLS:(f + 1) * COLS]

        def load_group(w, grp):
            g = w * WIN + grp * 3
            rgbt = p_rgb.tile([P, 3 * COLS * 3], F32, tag="rgb")
            nc.sync.dma_start(
                rgbt[:].rearrange("p (f x) -> p f x", f=3),
                vin[g:g + 3].rearrange("f (r x) -> r f x", r=P))
            return rgbt

        def p1_frame(w, f, rgbt, k):
            S = st8[w]
            rgb3 = rgbt[:, k * COLS * 3:(k + 1) * COLS * 3].rearrange(
                "p (j c) -> p j c", c=3)
            yf = yslice(w, f)
            t2 = p_ftmp.tile([P, COLS], F16, tag="t2")
            nc.scalar.activation(yf, rgb3[:, :, 0], AF.Copy, bias=0.0, scale=a_r)
            nc.scalar.activation(t2[:], rgb3[:, :, 2], AF.Copy, bias=0.0, scale=a_b)
            nc.gpsimd.tensor_tensor(t2[:], t2[:], rgb3[:, :, 1], OP.add)
            nc.vector.tensor_tensor(yf, yf, t2[:], OP.add)       # fp16 2x
            sq = p_ftmp.tile([P, COLS], F16, tag="sq")
            if SQ_ON_DVE(f):
                nc.vector.tensor_tensor(sq[:], yf, yf, OP.mult)  # fp16 2x
            else:
                nc.scalar.activation(sq[:], yf, AF.Square)
            for lo, hi in BANKS:
                nc.tensor.matmul(S["acc_s"][:, lo:hi], ident[:], yf[:, lo:hi],
                                 start=(f == 0), stop=(f == WIN - 1))
            for lo, hi in BANKS:
                nc.tensor.matmul(S["acc_q"][:, lo:hi], ident[:], sq[:, lo:hi],
                                 start=(f == 0), stop=(f == WIN - 1))

        def p2(w):
            S = st8[w]
            # mean (fp16, for the subtract)
            nc.scalar.activation(S["mt"][:], S["acc_s"][:], AF.Copy,
                                 bias=0.0, scale=float(1.0 / WIN))
            # 15*m^2 = (sum/sqrt(15))^2, exact from psum
            msq = p_tmp.tile([P, COLS], F32, tag="msq")
            nc.scalar.activation(msq[:], S["acc_s"][:], AF.Square,
                                 scale=float(1.0 / np.sqrt(15.0)))
            vs = p_tmp.tile([P, COLS], F32, tag="vs")
            nc.vector.tensor_tensor(vs[:], S["acc_q"][:], msq[:], OP.subtract)
            # sigma' = sqrt(varsum/14); inv_s = 1/(sigma' + EPS/w1)
            nc.scalar.activation(vs[:], vs[:], AF.Sqrt,
                                 scale=float(1.0 / (WIN - 1)))
            nc.vector.tensor_scalar(vs[:], vs[:], float(EPS / w1), None, OP.add)
            scr = p_tmp.tile([P, COLS], F32, tag="msq")
            nc.vector.reciprocal_approx_accurate(vs[:], vs[:], scr[:])
            nc.vector.tensor_copy(S["ish"][:], vs[:])            # cast -> fp16

        def p3_group(w, grp):
            S = st8[w]
            f0 = grp * 3
            for f in range(f0, f0 + 3):
                nc.vector.tensor_tensor(yslice(w, f), yslice(w, f), S["mt"][:],
                                        OP.subtract)             # d, fp16 2x
            for f in range(f0, f0 + 3):
                nc.vector.tensor_tensor(yslice(w, f), yslice(w, f), S["ish"][:],
                                        OP.mult)                 # bg signed
            ych3 = S["yt"][:, f0 * COLS:(f0 + 3) * COLS].rearrange(
                "p (f j) -> p f j", f=3)
            nc.vector.tensor_reduce(
                S["mmt"][:, f0:f0 + 3], ych3, axis=mybir.AxisListType.X,
                op=OP.max, apply_absolute_value=True)            # max |bg|

        def p4(w):
            mmt = st8[w]["mmt"]
            nc.gpsimd.partition_all_reduce(
                mmt[:, 16:32], mmt[:, 0:16], 128, bass_isa.ReduceOp.max)
            nc.vector.reciprocal(mmt[:, 32:48], mmt[:, 16:32])

        def p5_group(w, grp):
            S = st8[w]
            mmt = S["mmt"]
            f0 = grp * 3
            for f in range(f0, f0 + 3):
                # |bg * inv_maxv| : ACT Abs with per-partition scale column
                nc.scalar.activation(
                    yslice(w, f), yslice(w, f), AF.Abs,
                    bias=0.0, scale=mmt[:, 32 + f:33 + f])
            g0 = w * WIN + f0
            nc.scalar.dma_start(
                vout[g0:g0 + 3].rearrange("f (r j) -> r f j", r=P),
                S["yt"][:, f0 * COLS:(f0 + 3) * COLS].rearrange(
                    "p (f j) -> p f j", f=3),
            )

        # ---- software-pipelined emission ----
        mk_state(0)
        for grp in range(5):
            rgbt = load_group(0, grp)
            for k in range(3):
                p1_frame(0, grp * 3 + k, rgbt, k)
        for w in range(NWIN_CORE):
            nxt = w + 1 if w + 1 < NWIN_CORE else None
            if nxt is not None:
                mk_state(nxt)
            p2(w)
            for grp in range(5):
                p3_group(w, grp)
                if nxt is not None and grp < 3:
                    rgbt = load_group(nxt, grp)
                    for k in range(3):
                        p1_frame(nxt, grp * 3 + k, rgbt, k)
            p4(w)
            for grp in range(5):
                p5_group(w, grp)
                if nxt is not None and grp < 2:
                    rgbt = load_group(nxt, 3 + grp)
                    for k in range(3):
                        p1_frame(nxt, (3 + grp) * 3 + k, rgbt, k)
            del st8[w]

    nc.compile()
    return nc


def _get_nc(scale: float):
    key = round(float(scale), 9)
    if key not in _BUILD_CACHE:
        _BUILD_CACHE[key] = _build(key)
    return _BUILD_CACHE[key]


def kernel(video: np.ndarray) -> np.ndarray:
    video = np.ascontiguousarray(np.asarray(video, dtype=np.float32))
    assert video.shape == (T, H, W, 3), video.shape
    scale = 1.0 / 255.0 if float(video.max()) > 1.0 else 1.0
    nc = _get_nc(scale)

    v = video.reshape(T, PIX * 3)
    shards = []
    for c in range(6):
        shards.append(v[c * FPC:(c + 1) * FPC])
    # cores 6,7: 3 real windows + last window repeated as pad
    shards.append(np.concatenate([v[360:405], v[390:405]], axis=0))
    shards.append(np.concatenate([v[405:450], v[435:450]], axis=0))

    ident = np.eye(P, dtype=np.float16)
    res = run_bass_kernel_spmd(
        nc, [{"video": s, "ident": ident} for s in shards], list(range(NCORES))
    )
    outs = [res.results[c]["out"].astype(np.float32) for c in range(NCORES)]
    full = np.concatenate(
        [o[:FPC] for o in outs[:6]] + [outs[6][:45], outs[7][:45]], axis=0
    )
    return full.reshape(T, 1, H, W)
